# revision 1
# baseline (speedup 1.0000x reference)
"""Trainium2 Bass kernel: CLUTRR-style GNN message passing (nn_CLUTRRV4).

Data-parallel across 8 NeuronCores. Samples are packed 4-per-group
(4 x 32 entity slots = 128 partitions); sample->group assignment is an
LPT bin-packing so that each group's VALID edges fit in EC=128 packed
edge columns (vs 256 naive), skipping all masked-edge compute.

All one-hot gather/scatter/rel matrices are precomputed on the host and
DMA'd once (they are step-invariant); nothing is generated on-chip.
Entity state S is fp16-only (tolerance 2e-2 >> fp16 error here).

Per step, per group: S is transposed (PE) to slot-major, src/tgt states
are gathered via one-hot matmuls, the message MLP layer 1 uses fixed
weight blocks (rel contribution via the 20-row band trick), messages are
scattered back with the edge-major one-hot, and the update MLP runs per
quad (4 groups) with N=512 matmuls. Emission is a software-pipelined
flat loop (modulo schedule) so the PE never waits on the DVE/Act
converts; PSUM is budgeted at exactly 8 banks.
"""
import sys
import numpy as np

if "/opt/trn_rl_repo" not in sys.path:
    sys.path.append("/opt/trn_rl_repo")

N_ENT, N_REL, D, E = 32, 20, 128, 64
N_STEPS = 8
N_CORES = 8
P = 128
EC = 128          # packed edge columns per group
GRP = 4           # samples per group


def _build_nc(G, n_steps):
    from concourse import bacc, mybir
    from concourse.tile import TileContext
    from concourse.masks import make_identity

    f32 = mybir.dt.float32
    f16 = mybir.dt.float16
    AF = mybir.ActivationFunctionType
    OP = mybir.AluOpType

    assert G % 4 == 0
    NQ = G // 4
    SLOTS = G * P

    nc = bacc.Bacc()

    def din(name, shape, dtype=f32):
        return nc.declare_dram_parameter(name, list(shape), dtype, isOutput=False)

    d_s0 = din("s0", (P, SLOTS), f16)
    d_oh = din("oh", (NQ, P, 12 * EC), f16)     # 4 groups x [ohs|oht|ohe]
    d_relt = din("relt", (P, NQ * EC), f16)
    d_indt = din("indt", (P, NQ * EC), f16)
    d_qoh = din("qoh", (P, G * 8), f16)
    d_wf16 = din("wf16", (P, 1920), f16)
    d_wf32 = din("wf32", (P, 280))
    d_cb2 = din("cb2", (20, 1))
    d_out = nc.declare_dram_parameter("out", [20, G * GRP], f32, isOutput=True)

    with TileContext(nc) as tc:
        with (
            tc.tile_pool(name="c", bufs=1) as cp,
            tc.tile_pool(name="w", bufs=4) as wp,
            tc.tile_pool(name="pGA", bufs=2, space="PSUM") as pGA,
            tc.tile_pool(name="pH1", bufs=2, space="PSUM") as pH1,
            tc.tile_pool(name="pMS", bufs=1, space="PSUM") as pMS,
            tc.tile_pool(name="pAG", bufs=1, space="PSUM") as pAG,
            tc.tile_pool(name="pUP", bufs=2, space="PSUM") as pUP,
        ):
            wf16 = cp.tile([P, 1920], f16, tag="wf16", name="wf16")
            nc.sync.dma_start(wf16[:], d_wf16[:])
            w1s = wf16[:, 0:256]
            w1t = wf16[:, 256:512]
            rt4 = wf16[:, 512:768]
            w2m = wf16[:, 768:1024]
            w1u = wf16[:, 1024:1536]
            w2u = wf16[:, 1536:1792]
            b2row = wf16[:, 1792:1920]
            wf32 = cp.tile([P, 280], f32, tag="wf32", name="wf32")
            nc.sync.dma_start(wf32[:], d_wf32[:])
            b1u = wf32[:, 0:2]
            b2u = wf32[:, 2:3]
            cw1 = wf32[:, 3:259]
            cb1 = wf32[:, 259:260]
            cw2 = wf32[:, 260:280]
            cb2 = cp.tile([20, 1], f32, tag="cb2", name="cb2")
            nc.sync.dma_start(cb2[:], d_cb2[:])

            ident = cp.tile([P, P], f16, tag="ident", name="ident")
            make_identity(nc, ident[:])

            # interleave S-quad and one-hot-quad DMAs so step-0 compute
            # never starves; rel/ind early (needed at h1/sc offsets)
            S = cp.tile([P, SLOTS], f16, tag="S", name="S")
            OH = cp.tile([P, G * 3 * EC], f16, tag="OH", name="OH")
            RELT = cp.tile([P, NQ * EC], f16, tag="RELT", name="RELT")
            INDT = cp.tile([P, NQ * EC], f16, tag="INDT", name="INDT")
            for q in range(NQ):
                nc.sync.dma_start(S[:, q * 512:(q + 1) * 512],
                                  d_s0[:, q * 512:(q + 1) * 512])
                nc.sync.dma_start(OH[:, q * 1536:(q + 1) * 1536], d_oh[q])
                if q == 0:
                    nc.sync.dma_start(RELT[:], d_relt[:])
                    nc.sync.dma_start(INDT[:], d_indt[:])
            qoh = cp.tile([P, G * 8], f16, tag="qoh", name="qoh")
            nc.sync.dma_start(qoh[:], d_qoh[:])

            outsb = cp.tile([20, G * GRP], f32, tag="outsb", name="outsb")

            def ohs(g):
                return OH[:, g * 384:g * 384 + EC]

            def oht(g):
                return OH[:, g * 384 + EC:g * 384 + 2 * EC]

            def ohe(g):
                return OH[:, g * 384 + 2 * EC:g * 384 + 3 * EC]

            mm = nc.tensor.matmul
            st_ = {}
            sts_t, gtb_t, h1g_t, msb_t, agb_t = {}, {}, {}, {}, {}

            # --- pipeline stages ------------------------------------------
            def st_stage(t, g):
                # slot-major S replica via DMA XBAR transpose (no PE/PSUM)
                t_ = wp.tile([P, P], f16, tag="sts", bufs=8, name="sts")
                nc.sync.dma_start_transpose(t_[:], S[:, g * P:(g + 1) * P])
                sts_t[g] = t_

            def ga_stage(t, g):
                gi = g % 2
                if gi == 0:
                    st_['gap'] = pGA.tile([P, 512], f32, tag="ga", name="gap")
                gp = st_['gap']
                mm(gp[:, gi * 256:(gi + 1) * 256],
                   lhsT=sts_t[g][:], rhs=OH[:, g * 384:g * 384 + 2 * EC],
                   start=True, stop=True)
                if gi == 1:
                    t_ = wp.tile([P, 512], f16, tag="gtb", name="gtb")
                    nc.vector.tensor_copy(t_[:], gp[:])
                    gtb_t[g // 2] = t_

            def h1_stage(t, g):
                q = g // 4
                gi = g % 2
                if gi == 0:
                    st_['h1p'] = pH1.tile([P, 512], f32, tag="h1", name="h1p")
                hp = st_['h1p']
                gtb = gtb_t[g // 2]
                goff = gi * 256
                rb = (g % 4) * 32
                for c in (0, 1):
                    o = hp[:, gi * 256 + c * EC:gi * 256 + (c + 1) * EC]
                    mm(o, lhsT=rt4[rb:rb + 20, c * P:(c + 1) * P],
                       rhs=RELT[rb:rb + 20, q * EC:(q + 1) * EC],
                       start=True, stop=False, tile_position=(rb, 0))
                    mm(o, lhsT=w1s[:, c * P:(c + 1) * P],
                       rhs=gtb[:, goff:goff + EC], start=False, stop=False)
                    mm(o, lhsT=w1t[:, c * P:(c + 1) * P],
                       rhs=gtb[:, goff + EC:goff + 2 * EC],
                       start=False, stop=True)
                if gi == 1:
                    t_ = wp.tile([P, 512], f16, tag="h1g", name="h1g")
                    nc.scalar.activation(t_[:], hp[:], AF.Gelu)
                    h1g_t[g // 2] = t_

            def ms_stage(t, g):
                j = g % 4
                if j == 0:
                    st_['msp'] = pMS.tile([P, 512], f32, tag="ms", name="msp")
                mp = st_['msp']
                h1g = h1g_t[g // 2]
                hoff = (g % 2) * 256
                o = mp[:, j * P:(j + 1) * P]
                for c in (0, 1):
                    mm(o, lhsT=h1g[:, hoff + c * P:hoff + (c + 1) * P],
                       rhs=w2m[:, c * P:(c + 1) * P],
                       start=(c == 0), stop=(c == 1))
                if j == 3:
                    t_ = wp.tile([P, 512], f16, tag="msb", name="msb")
                    nc.vector.tensor_copy(t_[:], mp[:])
                    msb_t[g // 4] = t_

            def sc_stage(t, g):
                q, j = divmod(g, 4)
                if j == 0:
                    st_['agp'] = pAG.tile([P, 512], f32, tag="ag", name="agp")
                ap_ = st_['agp']
                msb = msb_t[q]
                rb = j * 32
                o = ap_[:, j * P:(j + 1) * P]
                mm(o, lhsT=msb[:, j * P:(j + 1) * P], rhs=ohe(g),
                   start=True, stop=False)
                mm(o, lhsT=b2row[rb:rb + 1, :],
                   rhs=INDT[rb:rb + 1, q * EC:(q + 1) * EC],
                   start=False, stop=True, tile_position=(rb, 0))
                if j == 3:
                    t_ = wp.tile([P, 512], f16, tag="agb", name="agb")
                    nc.scalar.copy(t_[:], ap_[:])
                    agb_t[q] = t_

            def up_stage(t, w):
                q, ph = divmod(w, 4)
                if ph == 0:
                    h3a = pUP.tile([P, 512], f32, tag="up", name="h3a")
                    h3b = pUP.tile([P, 512], f32, tag="up", name="h3b")
                    st_['h3a'], st_['h3b'] = h3a, h3b
                    for mc, hb in ((0, h3a), (1, h3b)):
                        mm(hb[:], lhsT=w1u[:, mc * P:(mc + 1) * P],
                           rhs=S[:, q * 512:(q + 1) * 512],
                           start=True, stop=False)
                        mm(hb[:], lhsT=w1u[:, 256 + mc * P:256 + (mc + 1) * P],
                           rhs=agb_t[q][:], start=False, stop=True)
                elif ph == 1:
                    t_ = wp.tile([P, 1024], f16, tag="h3g", name="h3g")
                    nc.scalar.activation(t_[:, 0:512], st_['h3a'][:], AF.Gelu,
                                         bias=b1u[:, 0:1])
                    st_['h3g'] = t_
                elif ph == 2:
                    nc.scalar.activation(st_['h3g'][:, 512:1024],
                                         st_['h3b'][:], AF.Gelu,
                                         bias=b1u[:, 1:2])
                else:
                    sn = pUP.tile([P, 512], f32, tag="up", name="sn")
                    for kc in (0, 1):
                        mm(sn[:], lhsT=w2u[:, kc * P:(kc + 1) * P],
                           rhs=st_['h3g'][:, kc * 512:(kc + 1) * 512],
                           start=(kc == 0), stop=(kc == 1))
                    nc.vector.scalar_tensor_tensor(
                        out=S[:, q * 512:(q + 1) * 512], in0=sn[:],
                        scalar=b2u[:, 0:1], in1=S[:, q * 512:(q + 1) * 512],
                        op0=OP.add, op1=OP.add)

            # --- software-pipelined flat loop -----------------------------
            # Stages are emitted deepest-offset first within each unit so
            # that every read of a rotating tile precedes the alloc of the
            # generation that reuses its buffer (WAR legality).
            # In-unit order: stall-prone update mms go LAST (in-order PE:
            # a waiting sn mm would block every later mm in the unit), and
            # h1g gelus land early in the Act queue. WAR legality needs
            # ms < h1 < ga.
            assert G >= 28, "modulo-schedule offsets need G >= 28"
            offs = (24, 19, 14, 10, 6, 0)
            stages = (up_stage, sc_stage, ms_stage, h1_stage, ga_stage,
                      st_stage)
            total = n_steps * G + offs[0] + 4
            for U in range(total):
                for off, fn in zip(offs, stages):
                    v = U - off
                    if v < 0:
                        continue
                    t, g = divmod(v, G)
                    if t < n_steps:
                        fn(t, g)

            # --- classifier head ------------------------------------------
            nbatch = (G + 15) // 16
            for bq in range(nbatch):
                jn = min(16, G - bq * 16)
                qps = pGA.tile([P, 128], f32, tag="ga", name="qps")
                for j in range(jn):
                    g = bq * 16 + j
                    t_ = wp.tile([P, P], f16, tag="sts", bufs=8, name="csts")
                    nc.sync.dma_start_transpose(t_[:], S[:, g * P:(g + 1) * P])
                    mm(qps[:, j * 8:(j + 1) * 8], lhsT=t_[:],
                       rhs=qoh[:, g * 8:(g + 1) * 8], start=True, stop=True)
                qcat = wp.tile([P, 128], f32, tag="qcat", name="qcat")
                nc.vector.tensor_copy(qcat[:, 0:jn * 8], qps[:, 0:jn * 8])
                qv = qcat[:, 0:jn * 8].rearrange("p (g t f) -> p g t f",
                                                 t=2, f=4)
                hps = pMS.tile([P, 64], f32, tag="ms", name="hps")
                mm(hps[:, 0:jn * 4], lhsT=cw1[:, 0:P], rhs=qv[:, :, 0, :],
                   start=True, stop=False)
                mm(hps[:, 0:jn * 4], lhsT=cw1[:, P:256], rhs=qv[:, :, 1, :],
                   start=False, stop=True)
                hg = wp.tile([P, 64], f32, tag="hg", name="hg")
                nc.scalar.activation(hg[:, 0:jn * 4], hps[:, 0:jn * 4],
                                     AF.Gelu, bias=cb1[:, 0:1])
                ops_ = pAG.tile([20, 64], f32, tag="ag", name="ops")
                mm(ops_[:, 0:jn * 4], lhsT=cw2[:], rhs=hg[:, 0:jn * 4],
                   start=True, stop=True)
                nc.scalar.activation(outsb[:, bq * 64:bq * 64 + jn * 4],
                                     ops_[:, 0:jn * 4], AF.Identity,
                                     bias=cb2[:, 0:1])
            nc.sync.dma_start(d_out[:], outsb[:])

    nc.finalize()
    return nc


def _assign_groups(ne, n_groups):
    """LPT bin packing: samples -> groups of <=GRP samples, balancing edge
    counts. Returns (group_of_sample, slot_of_sample, max_load)."""
    import heapq
    B = ne.shape[0]
    order = np.argsort(-ne, kind="stable")
    loads = [0] * n_groups
    counts = [0] * n_groups
    gof = np.zeros(B, np.int64)
    sof = np.zeros(B, np.int64)
    hp = [(0, g) for g in range(n_groups)]
    heapq.heapify(hp)
    for s in order:
        while True:
            ld, g = heapq.heappop(hp)
            if counts[g] < GRP and ld == loads[g]:
                break
        gof[s] = g
        sof[s] = counts[g]
        counts[g] += 1
        loads[g] += int(ne[s])
        if counts[g] < GRP:
            heapq.heappush(hp, (loads[g], g))
    return gof, sof, max(loads)


def _host_prep(inputs, G=None):
    f, hh = np.float32, np.float16
    src = np.asarray(inputs["edge_src"], np.int64)
    rel = np.asarray(inputs["edge_rel"], np.int64)
    tgt = np.asarray(inputs["edge_tgt"], np.int64)
    ne = np.asarray(inputs["n_edges"], np.int64)
    qs = np.asarray(inputs["query_src"], np.int64)
    qt = np.asarray(inputs["query_tgt"], np.int64)
    B = src.shape[0]
    if G is None:
        bc = -(-B // N_CORES)
        G = max(28, (-(-bc // GRP) + 3) // 4 * 4)
    while True:
        gof, sof, maxload = _assign_groups(ne, G * N_CORES)
        if maxload <= EC:
            break
        G += 4
    NG = G * N_CORES
    NQ = G // 4

    oh = np.zeros((NG, P, 3 * EC), hh)
    relt = np.zeros((NG // 4, P, EC), hh)
    indt = np.zeros((NG // 4, P, EC), hh)
    qoh = np.zeros((NG, P, 8), hh)
    ecnt = np.zeros(NG, np.int64)
    for s in range(B):
        g = int(gof[s])
        so = int(sof[s]) * N_ENT
        k = int(ne[s])
        q4, b4 = divmod(g, 4)
        if k:
            e0 = int(ecnt[g])
            ecnt[g] += k
            idx = np.arange(e0, e0 + k)
            es, et, er = src[s, :k], tgt[s, :k], rel[s, :k]
            oh[g, so + es, idx] = 1
            oh[g, so + et, EC + idx] = 1
            oh[g, idx, 2 * EC + so + et] = 1
            relt[q4, b4 * 32 + er, idx] = 1
            np.add.at(indt, (q4, b4 * 32, so + et), np.float16(1.0))
        qoh[g, so + qs[s], sof[s]] = 1
        qoh[g, so + qt[s], 4 + sof[s]] = 1

    # shared weights
    ee = np.asarray(inputs["entity_embed"], f)
    W1 = np.asarray(inputs["msg_W1"], f)
    reltab = (np.asarray(inputs["rel_embed"], f) @ W1[128:256]
              + np.asarray(inputs["msg_b1"], f))
    rt4 = np.zeros((P, 256), f)
    b2row = np.zeros((P, P), f)
    for rb in (0, 32, 64, 96):
        rt4[rb:rb + 20] = reltab
        b2row[rb] = np.asarray(inputs["msg_b2"], f)
    w2m_ = np.asarray(inputs["msg_W2"], f)
    w1u_ = np.asarray(inputs["upd_W1"], f)
    w2u_ = np.asarray(inputs["upd_W2"], f)
    cw1_ = np.asarray(inputs["cls_W1"], f)

    wf16 = np.concatenate([
        W1[0:128], W1[256:384], rt4,
        np.concatenate([w2m_[0:128], w2m_[128:256]], axis=1),
        np.concatenate(
            [w1u_[0:128, 0:128], w1u_[0:128, 128:256],
             w1u_[128:256, 0:128], w1u_[128:256, 128:256]], axis=1),
        np.concatenate([w2u_[0:128], w2u_[128:256]], axis=1),
        b2row,
    ], axis=1).astype(hh)
    wf32 = np.concatenate([
        np.asarray(inputs["upd_b1"], f).reshape(2, 128).T,
        np.asarray(inputs["upd_b2"], f).reshape(128, 1),
        np.concatenate([cw1_[0:128], cw1_[128:256]], axis=1),
        np.asarray(inputs["cls_b1"], f).reshape(128, 1),
        np.asarray(inputs["cls_W2"], f),
    ], axis=1).astype(f)
    shared = {
        "s0": np.tile(ee.T, (1, GRP * G)).astype(hh),
        "wf16": wf16,
        "wf32": wf32,
        "cb2": np.asarray(inputs["cls_b2"], f).reshape(20, 1).copy(),
    }

    in_maps = []
    for c in range(N_CORES):
        gsl = slice(c * G, (c + 1) * G)
        qsl = slice(c * NQ, (c + 1) * NQ)
        m = dict(shared)
        m["oh"] = np.ascontiguousarray(
            oh[gsl].reshape(NQ, 4, P, 3 * EC).transpose(0, 2, 1, 3)
            .reshape(NQ, P, 12 * EC))
        m["relt"] = np.ascontiguousarray(
            relt[qsl].transpose(1, 0, 2).reshape(P, NQ * EC))
        m["indt"] = np.ascontiguousarray(
            indt[qsl].transpose(1, 0, 2).reshape(P, NQ * EC))
        m["qoh"] = np.ascontiguousarray(
            qoh[gsl].transpose(1, 0, 2).reshape(P, G * 8))
        in_maps.append(m)
    return in_maps, gof, sof, G


_CACHE = {}


def kernel(**inputs):
    B = np.asarray(inputs["edge_src"]).shape[0]
    in_maps, gof, sof, G = _host_prep(inputs)

    key = G
    if key not in _CACHE:
        _CACHE[key] = _build_nc(G, N_STEPS)
    nc = _CACHE[key]

    from concourse.bass_utils import run_bass_kernel_spmd
    res = run_bass_kernel_spmd(nc, in_maps, core_ids=list(range(N_CORES)))

    out = np.empty((B, N_REL), np.float32)
    allc = np.concatenate([r["out"].T for r in res.results], axis=0)
    # row index in allc: core * (G*4) + (g_local*4 + slot) = gof*4 + sof
    out[:, :] = allc[gof * 4 + sof]
    return np.ascontiguousarray(out)



# revision 18
# speedup vs baseline: 1.0862x; 1.0862x over previous
"""Trainium2 Bass kernel: CLUTRR-style GNN message passing (nn_CLUTRRV4).

Data-parallel across 8 NeuronCores. Samples are packed 4-per-group
(4 x 32 entity slots = 128 partitions); sample->group assignment is an
LPT bin-packing so that each group's VALID edges fit in EC=128 packed
edge columns (vs 256 naive), skipping all masked-edge compute.

All one-hot gather/scatter/rel matrices are precomputed on the host and
DMA'd once (they are step-invariant); nothing is generated on-chip.
Entity state S is fp16-only (tolerance 2e-2 >> fp16 error here).

Per step, per group: S is transposed (PE) to slot-major, src/tgt states
are gathered via one-hot matmuls, the message MLP layer 1 uses fixed
weight blocks (rel contribution via the 20-row band trick), messages are
scattered back with the edge-major one-hot, and the update MLP runs per
quad (4 groups) with N=512 matmuls. Emission is a software-pipelined
flat loop (modulo schedule) so the PE never waits on the DVE/Act
converts; PSUM is budgeted at exactly 8 banks.
"""
import sys
import numpy as np

if "/opt/trn_rl_repo" not in sys.path:
    sys.path.append("/opt/trn_rl_repo")

N_ENT, N_REL, D, E = 32, 20, 128, 64
N_STEPS = 8
N_CORES = 8
P = 128
EC = 128          # packed edge columns per group
GRP = 4           # samples per group
ALL_SYNC_TR = False      # bisect flag: all S-transposes via sync DMA
PAIRWIDE_H1 = True     # bisect flag: pair-wide N=256 h1 mms vs per-group
WIDE_B2 = False         # bisect flag: quad-wide K=1 b2 mm vs per-group


def _build_nc(G, n_steps):
    from concourse import bacc, mybir
    from concourse.tile import TileContext
    from concourse.masks import make_identity

    f32 = mybir.dt.float32
    f16 = mybir.dt.float16
    AF = mybir.ActivationFunctionType
    OP = mybir.AluOpType

    assert G % 4 == 0
    NQ = G // 4
    SLOTS = G * P

    nc = bacc.Bacc()

    def din(name, shape, dtype=f32):
        return nc.declare_dram_parameter(name, list(shape), dtype, isOutput=False)

    NP = G // 2
    d_s0 = din("s0", (P, SLOTS), f16)
    d_oh = din("oh", (NQ, P, 12 * EC), f16)     # 4 groups x [ohs|oht|ohe]
    d_relt2 = din("relt2", (P, NP * 256), f16)  # pair-wide rel one-hots
    d_indtw = din("indtw", (1, G * P), f16)     # per-slot valid-edge counts
    d_qoh = din("qoh", (P, G * 8), f16)
    d_wf16 = din("wf16", (P, 1920), f16)
    d_wf32 = din("wf32", (P, 280))
    d_cb2 = din("cb2", (20, 1))
    d_out = nc.declare_dram_parameter("out", [20, G * GRP], f32, isOutput=True)

    with TileContext(nc) as tc:
        with (
            tc.tile_pool(name="c", bufs=1) as cp,
            tc.tile_pool(name="w", bufs=4) as wp,
            tc.tile_pool(name="pGA", bufs=2, space="PSUM") as pGA,
            tc.tile_pool(name="pH1", bufs=1, space="PSUM") as pH1,
            tc.tile_pool(name="pMS", bufs=1, space="PSUM") as pMS,
            tc.tile_pool(name="pAG", bufs=1, space="PSUM") as pAG,
            tc.tile_pool(name="pUP", bufs=2, space="PSUM") as pUP,
            tc.tile_pool(name="pTR", bufs=1, space="PSUM") as pTR,
        ):
            wf16 = cp.tile([P, 1920], f16, tag="wf16", name="wf16")
            nc.sync.dma_start(wf16[:], d_wf16[:])
            w1s = wf16[:, 0:256]
            w1t = wf16[:, 256:512]
            rt4 = wf16[:, 512:768]
            w2m = wf16[:, 768:1024]
            w1u = wf16[:, 1024:1536]
            w2u = wf16[:, 1536:1792]
            b2row = wf16[:, 1792:1920]
            wf32 = cp.tile([P, 280], f32, tag="wf32", name="wf32")
            nc.sync.dma_start(wf32[:], d_wf32[:])
            b1u = wf32[:, 0:2]
            b2u = wf32[:, 2:3]
            cw1 = wf32[:, 3:259]
            cb1 = wf32[:, 259:260]
            cw2 = wf32[:, 260:280]
            cb2 = cp.tile([20, 1], f32, tag="cb2", name="cb2")
            nc.sync.dma_start(cb2[:], d_cb2[:])

            ident = cp.tile([P, P], f16, tag="ident", name="ident")
            make_identity(nc, ident[:])

            # interleave S-quad and one-hot-quad DMAs so step-0 compute
            # never starves; rel/ind early (needed at h1/sc offsets)
            S = cp.tile([P, SLOTS], f16, tag="S", name="S")
            OH = cp.tile([P, G * 3 * EC], f16, tag="OH", name="OH")
            RELT2 = cp.tile([P, NP * 256], f16, tag="RELT2", name="RELT2")
            INDTW = cp.tile([1, G * P], f16, tag="INDTW", name="INDTW")
            for q in range(NQ):
                nc.sync.dma_start(S[:, q * 512:(q + 1) * 512],
                                  d_s0[:, q * 512:(q + 1) * 512])
                nc.sync.dma_start(OH[:, q * 1536:(q + 1) * 1536], d_oh[q])
                if q == 0:
                    nc.sync.dma_start(RELT2[:], d_relt2[:])
                    nc.sync.dma_start(INDTW[:], d_indtw[:])
            qoh = cp.tile([P, G * 8], f16, tag="qoh", name="qoh")
            nc.sync.dma_start(qoh[:], d_qoh[:])

            outsb = cp.tile([20, G * GRP], f32, tag="outsb", name="outsb")

            def ohs(g):
                return OH[:, g * 384:g * 384 + EC]

            def oht(g):
                return OH[:, g * 384 + EC:g * 384 + 2 * EC]

            def ohe(g):
                return OH[:, g * 384 + 2 * EC:g * 384 + 3 * EC]

            mm = nc.tensor.matmul
            st_ = {}
            sts_t, gtb_t, h1g_t, msb_t, agb_t = {}, {}, {}, {}, {}

            # --- pipeline stages ------------------------------------------
            def st_stage(t, g):
                # slot-major S replica. Alternate DMA XBAR transpose (sync
                # queue) with PE-transpose (+DVE copy) so neither the sync
                # queue nor any one engine saturates on transposes.
                t_ = wp.tile([P, P], f16, tag="sts", bufs=8, name="sts")
                if g % 2 == 0 or ALL_SYNC_TR:
                    nc.sync.dma_start_transpose(t_[:], S[:, g * P:(g + 1) * P])
                else:
                    tp = pTR.tile([P, P], f16, tag="tr", name="tp")
                    mm(tp[:], lhsT=S[:, g * P:(g + 1) * P], rhs=ident[:],
                       is_transpose=True, start=True, stop=True)
                    nc.vector.tensor_copy(t_[:], tp[:])
                sts_t[g] = t_

            def ga_stage(t, g):
                gi = g % 2
                if gi == 0:
                    st_['gap'] = pGA.tile([P, 512], f32, tag="ga", name="gap")
                gp = st_['gap']
                mm(gp[:, gi * 256:(gi + 1) * 256],
                   lhsT=sts_t[g][:], rhs=OH[:, g * 384:g * 384 + 2 * EC],
                   start=True, stop=True)
                if gi == 1:
                    t_ = wp.tile([P, 512], f16, tag="gtb", name="gtb")
                    nc.vector.tensor_copy(t_[:], gp[:])
                    gtb_t[g // 2] = t_

            def h1_stage(t, g):
                # pair-wide: 6 mms of N=256 per 2 groups (vs 12 of N=128).
                # gtb layout is [g0s g0t g1s g1t]; strided AP views pick
                # (g0s,g1s) resp (g0t,g1t) as the 256-col moving operand.
                if g % 2 == 0:
                    return
                p = g // 2
                rb = (p % 2) * 32
                hp = pH1.tile([P, 512], f32, tag="h1", name="h1p")
                gtb = gtb_t[p]
                if PAIRWIDE_H1:
                    gtb4 = gtb[:].rearrange("p (g s e) -> p g s e",
                                            g=2, s=2, e=EC)
                    for c in (0, 1):
                        o = hp[:, c * 256:(c + 1) * 256]
                        mm(o, lhsT=rt4[rb:rb + 20, c * P:(c + 1) * P],
                           rhs=RELT2[rb:rb + 20, p * 256:(p + 1) * 256],
                           start=True, stop=False, tile_position=(rb, 0))
                        mm(o, lhsT=w1s[:, c * P:(c + 1) * P],
                           rhs=gtb4[:, :, 0, :], start=False, stop=False)
                        mm(o, lhsT=w1t[:, c * P:(c + 1) * P],
                           rhs=gtb4[:, :, 1, :], start=False, stop=True)
                else:
                    for c in (0, 1):
                        for m in (0, 1):
                            o = hp[:, c * 256 + m * EC:c * 256 + (m + 1) * EC]
                            mm(o, lhsT=rt4[rb:rb + 20, c * P:(c + 1) * P],
                               rhs=RELT2[rb:rb + 20,
                                         p * 256 + m * EC:p * 256 + (m + 1) * EC],
                               start=True, stop=False, tile_position=(rb, 0))
                            mm(o, lhsT=w1s[:, c * P:(c + 1) * P],
                               rhs=gtb[:, m * 256:m * 256 + EC],
                               start=False, stop=False)
                            mm(o, lhsT=w1t[:, c * P:(c + 1) * P],
                               rhs=gtb[:, m * 256 + EC:(m + 1) * 256],
                               start=False, stop=True)
                t_ = wp.tile([P, 512], f16, tag="h1g", name="h1g")
                nc.scalar.activation(t_[:], hp[:], AF.Gelu)
                h1g_t[p] = t_

            def ms_stage(t, g):
                j = g % 4
                if j == 0:
                    st_['msp'] = pMS.tile([P, 512], f32, tag="ms", name="msp")
                mp = st_['msp']
                h1g = h1g_t[g // 2]
                m = g % 2
                o = mp[:, j * P:(j + 1) * P]
                for c in (0, 1):
                    mm(o, lhsT=h1g[:, c * 256 + m * P:c * 256 + (m + 1) * P],
                       rhs=w2m[:, c * P:(c + 1) * P],
                       start=(c == 0), stop=(c == 1))
                if j == 3:
                    t_ = wp.tile([P, 512], f16, tag="msb", name="msb")
                    nc.vector.tensor_copy(t_[:], mp[:])
                    msb_t[g // 4] = t_

            def sc_stage(t, g):
                q, j = divmod(g, 4)
                if j == 0:
                    st_['agp'] = pAG.tile([P, 512], f32, tag="ag", name="agp")
                ap_ = st_['agp']
                msb = msb_t[q]
                o = ap_[:, j * P:(j + 1) * P]
                if WIDE_B2:
                    mm(o, lhsT=msb[:, j * P:(j + 1) * P], rhs=ohe(g),
                       start=True, stop=False, skip_group_check=True)
                    if j == 3:
                        # quad-wide K=1 mm adds msg_b2 * per-slot edge count
                        mm(ap_[:, 0:512], lhsT=b2row[0:1, :],
                           rhs=INDTW[0:1, q * 512:(q + 1) * 512],
                           start=False, stop=True, skip_group_check=True)
                else:
                    mm(o, lhsT=msb[:, j * P:(j + 1) * P], rhs=ohe(g),
                       start=True, stop=False)
                    mm(o, lhsT=b2row[0:1, :],
                       rhs=INDTW[0:1, g * P:(g + 1) * P],
                       start=False, stop=True)
                if j == 3:
                    t_ = wp.tile([P, 512], f16, tag="agb", name="agb")
                    nc.scalar.copy(t_[:], ap_[:])
                    agb_t[q] = t_

            def up_stage(t, w):
                q, ph = divmod(w, 4)
                if ph == 0:
                    h3a = pUP.tile([P, 512], f32, tag="up", name="h3a")
                    h3b = pUP.tile([P, 512], f32, tag="up", name="h3b")
                    st_['h3a'], st_['h3b'] = h3a, h3b
                    for mc, hb in ((0, h3a), (1, h3b)):
                        mm(hb[:], lhsT=w1u[:, mc * P:(mc + 1) * P],
                           rhs=S[:, q * 512:(q + 1) * 512],
                           start=True, stop=False)
                        mm(hb[:], lhsT=w1u[:, 256 + mc * P:256 + (mc + 1) * P],
                           rhs=agb_t[q][:], start=False, stop=True)
                elif ph == 1:
                    t_ = wp.tile([P, 1024], f16, tag="h3g", name="h3g")
                    nc.scalar.activation(t_[:, 0:512], st_['h3a'][:], AF.Gelu,
                                         bias=b1u[:, 0:1])
                    st_['h3g'] = t_
                elif ph == 2:
                    nc.scalar.activation(st_['h3g'][:, 512:1024],
                                         st_['h3b'][:], AF.Gelu,
                                         bias=b1u[:, 1:2])
                else:
                    sn = pUP.tile([P, 512], f32, tag="up", name="sn")
                    for kc in (0, 1):
                        mm(sn[:], lhsT=w2u[:, kc * P:(kc + 1) * P],
                           rhs=st_['h3g'][:, kc * 512:(kc + 1) * 512],
                           start=(kc == 0), stop=(kc == 1))
                    nc.vector.scalar_tensor_tensor(
                        out=S[:, q * 512:(q + 1) * 512], in0=sn[:],
                        scalar=b2u[:, 0:1], in1=S[:, q * 512:(q + 1) * 512],
                        op0=OP.add, op1=OP.add)

            # --- software-pipelined flat loop -----------------------------
            # Stages are emitted deepest-offset first within each unit so
            # that every read of a rotating tile precedes the alloc of the
            # generation that reuses its buffer (WAR legality).
            # In-unit order: stall-prone update mms go LAST (in-order PE:
            # a waiting sn mm would block every later mm in the unit), and
            # h1g gelus land early in the Act queue. WAR legality needs
            # ms < h1 < ga.
            assert G >= 28, "modulo-schedule offsets need G >= 28"
            offs = (24, 19, 14, 10, 6, 0)
            stages = (up_stage, sc_stage, ms_stage, h1_stage, ga_stage,
                      st_stage)
            total = n_steps * G + offs[0] + 4
            for U in range(total):
                for off, fn in zip(offs, stages):
                    v = U - off
                    if v < 0:
                        continue
                    t, g = divmod(v, G)
                    if t < n_steps:
                        fn(t, g)

            # --- classifier head ------------------------------------------
            nbatch = (G + 15) // 16
            for bq in range(nbatch):
                jn = min(16, G - bq * 16)
                qps = pGA.tile([P, 128], f32, tag="ga", name="qps")
                for j in range(jn):
                    g = bq * 16 + j
                    t_ = wp.tile([P, P], f16, tag="sts", bufs=8, name="csts")
                    if j % 2 == 0:
                        nc.sync.dma_start_transpose(t_[:],
                                                    S[:, g * P:(g + 1) * P])
                    else:
                        tp = pTR.tile([P, P], f16, tag="tr", name="ctp")
                        mm(tp[:], lhsT=S[:, g * P:(g + 1) * P], rhs=ident[:],
                           is_transpose=True, start=True, stop=True)
                        nc.vector.tensor_copy(t_[:], tp[:])
                    mm(qps[:, j * 8:(j + 1) * 8], lhsT=t_[:],
                       rhs=qoh[:, g * 8:(g + 1) * 8], start=True, stop=True)
                qcat = wp.tile([P, 128], f32, tag="qcat", name="qcat")
                nc.vector.tensor_copy(qcat[:, 0:jn * 8], qps[:, 0:jn * 8])
                qv = qcat[:, 0:jn * 8].rearrange("p (g t f) -> p g t f",
                                                 t=2, f=4)
                hps = pMS.tile([P, 64], f32, tag="ms", name="hps")
                mm(hps[:, 0:jn * 4], lhsT=cw1[:, 0:P], rhs=qv[:, :, 0, :],
                   start=True, stop=False)
                mm(hps[:, 0:jn * 4], lhsT=cw1[:, P:256], rhs=qv[:, :, 1, :],
                   start=False, stop=True)
                hg = wp.tile([P, 64], f32, tag="hg", name="hg")
                nc.scalar.activation(hg[:, 0:jn * 4], hps[:, 0:jn * 4],
                                     AF.Gelu, bias=cb1[:, 0:1])
                ops_ = pAG.tile([20, 64], f32, tag="ag", name="ops")
                mm(ops_[:, 0:jn * 4], lhsT=cw2[:], rhs=hg[:, 0:jn * 4],
                   start=True, stop=True)
                nc.scalar.activation(outsb[:, bq * 64:bq * 64 + jn * 4],
                                     ops_[:, 0:jn * 4], AF.Identity,
                                     bias=cb2[:, 0:1])
            nc.sync.dma_start(d_out[:], outsb[:])

    nc.finalize()
    return nc


def _assign_groups(ne, n_groups):
    """LPT bin packing: samples -> groups of <=GRP samples, balancing edge
    counts. Returns (group_of_sample, slot_of_sample, max_load)."""
    import heapq
    B = ne.shape[0]
    order = np.argsort(-ne, kind="stable")
    loads = [0] * n_groups
    counts = [0] * n_groups
    gof = np.zeros(B, np.int64)
    sof = np.zeros(B, np.int64)
    hp = [(0, g) for g in range(n_groups)]
    heapq.heapify(hp)
    for s in order:
        while True:
            ld, g = heapq.heappop(hp)
            if counts[g] < GRP and ld == loads[g]:
                break
        gof[s] = g
        sof[s] = counts[g]
        counts[g] += 1
        loads[g] += int(ne[s])
        if counts[g] < GRP:
            heapq.heappush(hp, (loads[g], g))
    return gof, sof, max(loads)


def _host_prep(inputs, G=None):
    f, hh = np.float32, np.float16
    src = np.asarray(inputs["edge_src"], np.int64)
    rel = np.asarray(inputs["edge_rel"], np.int64)
    tgt = np.asarray(inputs["edge_tgt"], np.int64)
    ne = np.asarray(inputs["n_edges"], np.int64)
    qs = np.asarray(inputs["query_src"], np.int64)
    qt = np.asarray(inputs["query_tgt"], np.int64)
    B = src.shape[0]
    if G is None:
        bc = -(-B // N_CORES)
        G = max(28, (-(-bc // GRP) + 3) // 4 * 4)
    while True:
        gof, sof, maxload = _assign_groups(ne, G * N_CORES)
        if maxload <= EC:
            break
        G += 4
    NG = G * N_CORES
    NQ = G // 4
    NP = G // 2

    oh = np.zeros((NG, P, 3 * EC), hh)
    relt2 = np.zeros((NG // 2, P, 256), hh)
    indtw = np.zeros((1, NG * P), hh)
    qoh = np.zeros((NG, P, 8), hh)
    ecnt = np.zeros(NG, np.int64)
    for s in range(B):
        g = int(gof[s])
        so = int(sof[s]) * N_ENT
        k = int(ne[s])
        pg, m2 = divmod(g, 2)
        rb = (pg % 2) * 32
        if k:
            e0 = int(ecnt[g])
            ecnt[g] += k
            idx = np.arange(e0, e0 + k)
            es, et, er = src[s, :k], tgt[s, :k], rel[s, :k]
            oh[g, so + es, idx] = 1
            oh[g, so + et, EC + idx] = 1
            oh[g, idx, 2 * EC + so + et] = 1
            relt2[pg, rb + er, m2 * EC + idx] = 1
            np.add.at(indtw, (0, g * P + so + et), np.float16(1.0))
        qoh[g, so + qs[s], sof[s]] = 1
        qoh[g, so + qt[s], 4 + sof[s]] = 1

    # shared weights
    ee = np.asarray(inputs["entity_embed"], f)
    W1 = np.asarray(inputs["msg_W1"], f)
    reltab = (np.asarray(inputs["rel_embed"], f) @ W1[128:256]
              + np.asarray(inputs["msg_b1"], f))
    rt4 = np.zeros((P, 256), f)
    b2row = np.zeros((P, P), f)
    for rb in (0, 32, 64, 96):
        rt4[rb:rb + 20] = reltab
        b2row[rb] = np.asarray(inputs["msg_b2"], f)
    w2m_ = np.asarray(inputs["msg_W2"], f)
    w1u_ = np.asarray(inputs["upd_W1"], f)
    w2u_ = np.asarray(inputs["upd_W2"], f)
    cw1_ = np.asarray(inputs["cls_W1"], f)

    wf16 = np.concatenate([
        W1[0:128], W1[256:384], rt4,
        np.concatenate([w2m_[0:128], w2m_[128:256]], axis=1),
        np.concatenate(
            [w1u_[0:128, 0:128], w1u_[0:128, 128:256],
             w1u_[128:256, 0:128], w1u_[128:256, 128:256]], axis=1),
        np.concatenate([w2u_[0:128], w2u_[128:256]], axis=1),
        b2row,
    ], axis=1).astype(hh)
    wf32 = np.concatenate([
        np.asarray(inputs["upd_b1"], f).reshape(2, 128).T,
        np.asarray(inputs["upd_b2"], f).reshape(128, 1),
        np.concatenate([cw1_[0:128], cw1_[128:256]], axis=1),
        np.asarray(inputs["cls_b1"], f).reshape(128, 1),
        np.asarray(inputs["cls_W2"], f),
    ], axis=1).astype(f)
    shared = {
        "s0": np.tile(ee.T, (1, GRP * G)).astype(hh),
        "wf16": wf16,
        "wf32": wf32,
        "cb2": np.asarray(inputs["cls_b2"], f).reshape(20, 1).copy(),
    }

    in_maps = []
    for c in range(N_CORES):
        gsl = slice(c * G, (c + 1) * G)
        psl = slice(c * NP, (c + 1) * NP)
        m = dict(shared)
        m["oh"] = np.ascontiguousarray(
            oh[gsl].reshape(NQ, 4, P, 3 * EC).transpose(0, 2, 1, 3)
            .reshape(NQ, P, 12 * EC))
        m["relt2"] = np.ascontiguousarray(
            relt2[psl].transpose(1, 0, 2).reshape(P, NP * 256))
        m["indtw"] = np.ascontiguousarray(
            indtw[:, c * G * P:(c + 1) * G * P])
        m["qoh"] = np.ascontiguousarray(
            qoh[gsl].transpose(1, 0, 2).reshape(P, G * 8))
        in_maps.append(m)
    return in_maps, gof, sof, G


_CACHE = {}


def kernel(**inputs):
    B = np.asarray(inputs["edge_src"]).shape[0]
    in_maps, gof, sof, G = _host_prep(inputs)

    key = G
    if key not in _CACHE:
        _CACHE[key] = _build_nc(G, N_STEPS)
    nc = _CACHE[key]

    from concourse.bass_utils import run_bass_kernel_spmd
    res = run_bass_kernel_spmd(nc, in_maps, core_ids=list(range(N_CORES)))

    out = np.empty((B, N_REL), np.float32)
    allc = np.concatenate([r["out"].T for r in res.results], axis=0)
    # row index in allc: core * (G*4) + (g_local*4 + slot) = gof*4 + sof
    out[:, :] = allc[gof * 4 + sof]
    return np.ascontiguousarray(out)



# revision 35
# speedup vs baseline: 1.2764x; 1.1750x over previous
"""Trainium2 Bass kernel: CLUTRR-style GNN message passing (nn_CLUTRRV4).

Data-parallel across 8 NeuronCores. Samples are packed 4-per-group
(4 x 32 entity slots = 128 partitions); sample->group assignment is an
LPT bin-packing so that each group's VALID edges fit in EC=128 packed
edge columns (vs 256 naive), skipping all masked-edge compute.

All one-hot gather/scatter/rel matrices are precomputed on the host and
DMA'd once (they are step-invariant); nothing is generated on-chip.
Entity state S is fp16-only (tolerance 2e-2 >> fp16 error here).

Per step, per group: S is transposed (PE) to slot-major, src/tgt states
are gathered via one-hot matmuls, the message MLP layer 1 uses fixed
weight blocks (rel contribution via the 20-row band trick), messages are
scattered back with the edge-major one-hot, and the update MLP runs per
quad (4 groups) with N=512 matmuls. Emission is a software-pipelined
flat loop (modulo schedule) so the PE never waits on the DVE/Act
converts; PSUM is budgeted at exactly 8 banks.
"""
import sys
import numpy as np

if "/opt/trn_rl_repo" not in sys.path:
    sys.path.append("/opt/trn_rl_repo")

N_ENT, N_REL, D, E = 32, 20, 128, 64
N_STEPS = 8
N_CORES = 8
P = 128
EC = 128          # packed edge columns per group
GRP = 4           # samples per group
STT_AGB = True
REL_BANDS = False



def _build_nc(G, n_steps):
    from concourse import bacc, mybir
    from concourse.tile import TileContext
    from concourse.masks import make_identity

    f32 = mybir.dt.float32
    f16 = mybir.dt.float16
    AF = mybir.ActivationFunctionType
    OP = mybir.AluOpType

    assert G % 4 == 0
    NQ = G // 4
    SLOTS = G * P

    nc = bacc.Bacc()

    def din(name, shape, dtype=f32):
        return nc.declare_dram_parameter(name, list(shape), dtype, isOutput=False)

    NP = G // 2
    d_s0 = din("s0", (P, SLOTS), f16)
    d_oh = din("oh", (NQ, P, 12 * EC), f16)     # 4 groups x [ohs|oht|ohe]
    d_relt2 = din("relt2", (P, NP * 256), f16)  # pair-wide rel one-hots
    d_cntb = din("cntb", (P, G * P), f16)       # per-slot edge counts, bcast
    d_qoh = din("qoh", (P, G * 8), f16)
    d_wf16 = din("wf16", (P, 1920), f16)
    d_wf32 = din("wf32", (P, 281))
    d_cb2 = din("cb2", (20, 1))
    d_out = nc.declare_dram_parameter("out", [20, G * GRP], f32, isOutput=True)

    with TileContext(nc) as tc:
        with (
            tc.tile_pool(name="c", bufs=1) as cp,
            tc.tile_pool(name="w", bufs=4) as wp,
            tc.tile_pool(name="pGA", bufs=2, space="PSUM") as pGA,
            tc.tile_pool(name="pH1", bufs=1, space="PSUM") as pH1,
            tc.tile_pool(name="pMS", bufs=1, space="PSUM") as pMS,
            tc.tile_pool(name="pAG", bufs=1, space="PSUM") as pAG,
            tc.tile_pool(name="pUP", bufs=2, space="PSUM") as pUP,
            tc.tile_pool(name="pTR", bufs=1, space="PSUM") as pTR,
        ):
            wf16 = cp.tile([P, 1920], f16, tag="wf16", name="wf16")
            nc.sync.dma_start(wf16[:], d_wf16[:])
            w1s = wf16[:, 0:256]
            w1t = wf16[:, 256:512]
            rt4 = wf16[:, 512:768]
            w2m = wf16[:, 768:1024]
            w1u = wf16[:, 1024:1536]
            w2u = wf16[:, 1536:1792]
            b2row = wf16[:, 1792:1920]
            wf32 = cp.tile([P, 281], f32, tag="wf32", name="wf32")
            nc.sync.dma_start(wf32[:], d_wf32[:])
            b1u = wf32[:, 0:2]
            b2u = wf32[:, 2:3]
            cw1 = wf32[:, 3:259]
            cb1 = wf32[:, 259:260]
            cw2 = wf32[:, 260:280]
            b2mc = wf32[:, 280:281]
            cb2 = cp.tile([20, 1], f32, tag="cb2", name="cb2")
            nc.sync.dma_start(cb2[:], d_cb2[:])

            ident = cp.tile([P, P], f16, tag="ident", name="ident")
            make_identity(nc, ident[:])

            # interleave S-quad and one-hot-quad DMAs so step-0 compute
            # never starves; rel/ind early (needed at h1/sc offsets)
            S = cp.tile([P, SLOTS], f16, tag="S", name="S")
            OH = cp.tile([P, G * 3 * EC], f16, tag="OH", name="OH")
            RELT2 = cp.tile([P, NP * 256], f16, tag="RELT2", name="RELT2")
            CNTB = cp.tile([P, G * P], f16, tag="CNTB", name="CNTB")
            for q in range(NQ):
                nc.sync.dma_start(S[:, q * 512:(q + 1) * 512],
                                  d_s0[:, q * 512:(q + 1) * 512])
                nc.sync.dma_start(OH[:, q * 1536:(q + 1) * 1536], d_oh[q])
                nc.sync.dma_start(CNTB[:, q * 512:(q + 1) * 512],
                                  d_cntb[:, q * 512:(q + 1) * 512])
                if q == 0:
                    nc.sync.dma_start(RELT2[:], d_relt2[:])
            qoh = cp.tile([P, G * 8], f16, tag="qoh", name="qoh")
            nc.sync.dma_start(qoh[:], d_qoh[:])

            outsb = cp.tile([20, G * GRP], f32, tag="outsb", name="outsb")

            def ohs(g):
                return OH[:, g * 384:g * 384 + EC]

            def oht(g):
                return OH[:, g * 384 + EC:g * 384 + 2 * EC]

            def ohe(g):
                return OH[:, g * 384 + 2 * EC:g * 384 + 3 * EC]

            mm = nc.tensor.matmul
            st_ = {}
            sts_t, gtb_t, h1g_t, msb_t, agb_t = {}, {}, {}, {}, {}

            # --- pipeline stages ------------------------------------------
            def st_stage(t, g):
                # slot-major S replica. Alternate DMA XBAR transpose (sync
                # queue) with PE-transpose (+DVE copy) so neither the sync
                # queue nor any one engine saturates on transposes.
                t_ = wp.tile([P, P], f16, tag="sts", bufs=8, name="sts")
                if g % 2 == 0:
                    nc.sync.dma_start_transpose(t_[:], S[:, g * P:(g + 1) * P])
                else:
                    tp = pTR.tile([P, P], f16, tag="tr", name="tp")
                    mm(tp[:], lhsT=S[:, g * P:(g + 1) * P], rhs=ident[:],
                       is_transpose=True, start=True, stop=True)
                    nc.vector.tensor_copy(t_[:], tp[:])
                sts_t[g] = t_

            def ga_stage(t, g):
                gi = g % 2
                if gi == 0:
                    st_['gap'] = pGA.tile([P, 512], f32, tag="ga", name="gap")
                gp = st_['gap']
                mm(gp[:, gi * 256:(gi + 1) * 256],
                   lhsT=sts_t[g][:], rhs=OH[:, g * 384:g * 384 + 2 * EC],
                   start=True, stop=True)
                if gi == 1:
                    t_ = wp.tile([P, 512], f16, tag="gtb", name="gtb")
                    nc.vector.tensor_copy(t_[:], gp[:])
                    gtb_t[g // 2] = t_

            def h1_stage(t, g):
                # pair-wide: 6 mms of N=256 per 2 groups (vs 12 of N=128).
                # gtb layout is [g0s g0t g1s g1t]; strided AP views pick
                # (g0s,g1s) resp (g0t,g1t) as the 256-col moving operand.
                if g % 2 == 0:
                    return
                p = g // 2
                rb = (p % 2) * 32
                hp = pH1.tile([P, 512], f32, tag="h1", name="h1p")
                gtb4 = gtb_t[p][:].rearrange("p (g s e) -> p g s e",
                                             g=2, s=2, e=EC)
                for c in (0, 1):
                    o = hp[:, c * 256:(c + 1) * 256]
                    mm(o, lhsT=rt4[rb:rb + 20, c * P:(c + 1) * P],
                       rhs=RELT2[rb:rb + 20, p * 256:(p + 1) * 256],
                       start=True, stop=False, tile_position=(rb, 0))
                    mm(o, lhsT=w1s[:, c * P:(c + 1) * P],
                       rhs=gtb4[:, :, 0, :], start=False, stop=False)
                    mm(o, lhsT=w1t[:, c * P:(c + 1) * P],
                       rhs=gtb4[:, :, 1, :], start=False, stop=True)
                t_ = wp.tile([P, 512], f16, tag="h1g", name="h1g")
                nc.scalar.activation(t_[:], hp[:], AF.Gelu)
                h1g_t[p] = t_

            def ms_stage(t, g):
                j = g % 4
                if j == 0:
                    st_['msp'] = pMS.tile([P, 512], f32, tag="ms", name="msp")
                mp = st_['msp']
                h1g = h1g_t[g // 2]
                m = g % 2
                o = mp[:, j * P:(j + 1) * P]
                for c in (0, 1):
                    mm(o, lhsT=h1g[:, c * 256 + m * P:c * 256 + (m + 1) * P],
                       rhs=w2m[:, c * P:(c + 1) * P],
                       start=(c == 0), stop=(c == 1))
                if j == 3:
                    t_ = wp.tile([P, 512], f16, tag="msb", name="msb")
                    nc.vector.tensor_copy(t_[:], mp[:])
                    msb_t[g // 4] = t_

            def sc_stage(t, g):
                q, j = divmod(g, 4)
                if j == 0:
                    st_['agp'] = pAG.tile([P, 512], f32, tag="ag", name="agp")
                ap_ = st_['agp']
                msb = msb_t[q]
                o = ap_[:, j * P:(j + 1) * P]
                mm(o, lhsT=msb[:, j * P:(j + 1) * P], rhs=ohe(g),
                   start=True, stop=True)
                if j == 3:
                    # agb = agg + msg_b2 * per-slot valid-edge count, folded
                    # into the PSUM->SBUF copy (no PE b2 matmuls)
                    t_ = wp.tile([P, 512], f16, tag="agb", name="agb")
                    if STT_AGB:
                        nc.vector.scalar_tensor_tensor(
                            out=t_[:], in0=ap_[:], scalar=1.0,
                            in1=CNTB[:, q * 512:(q + 1) * 512],
                            op0=OP.mult, op1=OP.add)
                    else:
                        nc.scalar.copy(t_[:], ap_[:])
                    agb_t[q] = t_

            def up_stage(t, w):
                q, ph = divmod(w, 4)
                if ph == 0:
                    h3a = pUP.tile([P, 512], f32, tag="up", name="h3a")
                    h3b = pUP.tile([P, 512], f32, tag="up", name="h3b")
                    st_['h3a'], st_['h3b'] = h3a, h3b
                    for mc, hb in ((0, h3a), (1, h3b)):
                        mm(hb[:], lhsT=w1u[:, mc * P:(mc + 1) * P],
                           rhs=S[:, q * 512:(q + 1) * 512],
                           start=True, stop=False)
                        mm(hb[:], lhsT=w1u[:, 256 + mc * P:256 + (mc + 1) * P],
                           rhs=agb_t[q][:], start=False, stop=True)
                elif ph == 1:
                    t_ = wp.tile([P, 1024], f16, tag="h3g", name="h3g")
                    nc.scalar.activation(t_[:, 0:512], st_['h3a'][:], AF.Gelu,
                                         bias=b1u[:, 0:1])
                    st_['h3g'] = t_
                elif ph == 2:
                    nc.scalar.activation(st_['h3g'][:, 512:1024],
                                         st_['h3b'][:], AF.Gelu,
                                         bias=b1u[:, 1:2])
                else:
                    sn = pUP.tile([P, 512], f32, tag="up", name="sn")
                    for kc in (0, 1):
                        mm(sn[:], lhsT=w2u[:, kc * P:(kc + 1) * P],
                           rhs=st_['h3g'][:, kc * 512:(kc + 1) * 512],
                           start=(kc == 0), stop=(kc == 1))
                    nc.vector.scalar_tensor_tensor(
                        out=S[:, q * 512:(q + 1) * 512], in0=sn[:],
                        scalar=b2u[:, 0:1], in1=S[:, q * 512:(q + 1) * 512],
                        op0=OP.add, op1=OP.add)

            # --- software-pipelined flat loop -----------------------------
            # Stages are emitted deepest-offset first within each unit so
            # that every read of a rotating tile precedes the alloc of the
            # generation that reuses its buffer (WAR legality).
            # In-unit order: stall-prone update mms go LAST (in-order PE:
            # a waiting sn mm would block every later mm in the unit), and
            # h1g gelus land early in the Act queue. WAR legality needs
            # ms < h1 < ga.
            assert G >= 28, "modulo-schedule offsets need G >= 28"
            offs = (24, 19, 14, 10, 6, 0)
            stages = (up_stage, sc_stage, ms_stage, h1_stage, ga_stage,
                      st_stage)
            total = n_steps * G + offs[0] + 4
            for U in range(total):
                for off, fn in zip(offs, stages):
                    v = U - off
                    if v < 0:
                        continue
                    t, g = divmod(v, G)
                    if t < n_steps:
                        fn(t, g)

            # --- classifier head ------------------------------------------
            nbatch = (G + 15) // 16
            for bq in range(nbatch):
                jn = min(16, G - bq * 16)
                qps = pGA.tile([P, 128], f32, tag="ga", name="qps")
                for j in range(jn):
                    g = bq * 16 + j
                    t_ = wp.tile([P, P], f16, tag="sts", bufs=8, name="csts")
                    if j % 2 == 0:
                        nc.sync.dma_start_transpose(t_[:],
                                                    S[:, g * P:(g + 1) * P])
                    else:
                        tp = pTR.tile([P, P], f16, tag="tr", name="ctp")
                        mm(tp[:], lhsT=S[:, g * P:(g + 1) * P], rhs=ident[:],
                           is_transpose=True, start=True, stop=True)
                        nc.vector.tensor_copy(t_[:], tp[:])
                    mm(qps[:, j * 8:(j + 1) * 8], lhsT=t_[:],
                       rhs=qoh[:, g * 8:(g + 1) * 8], start=True, stop=True)
                qcat = wp.tile([P, 128], f32, tag="qcat", name="qcat")
                nc.vector.tensor_copy(qcat[:, 0:jn * 8], qps[:, 0:jn * 8])
                qv = qcat[:, 0:jn * 8].rearrange("p (g t f) -> p g t f",
                                                 t=2, f=4)
                hps = pMS.tile([P, 64], f32, tag="ms", name="hps")
                mm(hps[:, 0:jn * 4], lhsT=cw1[:, 0:P], rhs=qv[:, :, 0, :],
                   start=True, stop=False)
                mm(hps[:, 0:jn * 4], lhsT=cw1[:, P:256], rhs=qv[:, :, 1, :],
                   start=False, stop=True)
                hg = wp.tile([P, 64], f32, tag="hg", name="hg")
                nc.scalar.activation(hg[:, 0:jn * 4], hps[:, 0:jn * 4],
                                     AF.Gelu, bias=cb1[:, 0:1])
                ops_ = pAG.tile([20, 64], f32, tag="ag", name="ops")
                mm(ops_[:, 0:jn * 4], lhsT=cw2[:], rhs=hg[:, 0:jn * 4],
                   start=True, stop=True)
                nc.scalar.activation(outsb[:, bq * 64:bq * 64 + jn * 4],
                                     ops_[:, 0:jn * 4], AF.Identity,
                                     bias=cb2[:, 0:1])
            nc.sync.dma_start(d_out[:], outsb[:])

    nc.finalize()
    return nc


def _assign_groups(ne, n_groups):
    """LPT bin packing: samples -> groups of <=GRP samples, balancing edge
    counts. Returns (group_of_sample, slot_of_sample, max_load)."""
    import heapq
    B = ne.shape[0]
    order = np.argsort(-ne, kind="stable")
    loads = [0] * n_groups
    counts = [0] * n_groups
    gof = np.zeros(B, np.int64)
    sof = np.zeros(B, np.int64)
    hp = [(0, g) for g in range(n_groups)]
    heapq.heapify(hp)
    for s in order:
        while True:
            ld, g = heapq.heappop(hp)
            if counts[g] < GRP and ld == loads[g]:
                break
        gof[s] = g
        sof[s] = counts[g]
        counts[g] += 1
        loads[g] += int(ne[s])
        if counts[g] < GRP:
            heapq.heappush(hp, (loads[g], g))
    return gof, sof, max(loads)


def _host_prep(inputs, G=None):
    f, hh = np.float32, np.float16
    src = np.asarray(inputs["edge_src"], np.int64)
    rel = np.asarray(inputs["edge_rel"], np.int64)
    tgt = np.asarray(inputs["edge_tgt"], np.int64)
    ne = np.asarray(inputs["n_edges"], np.int64)
    qs = np.asarray(inputs["query_src"], np.int64)
    qt = np.asarray(inputs["query_tgt"], np.int64)
    B = src.shape[0]
    if G is None:
        bc = -(-B // N_CORES)
        G = max(28, (-(-bc // GRP) + 3) // 4 * 4)
    while True:
        gof, sof, maxload = _assign_groups(ne, G * N_CORES)
        if maxload <= EC:
            break
        G += 4
    NG = G * N_CORES
    NQ = G // 4
    NP = G // 2

    oh = np.zeros((NG, P, 3 * EC), hh)
    relt2 = np.zeros((NG // 2, P, 256), hh)
    cnt = np.zeros(NG * P, np.float32)
    qoh = np.zeros((NG, P, 8), hh)
    ecnt = np.zeros(NG, np.int64)
    for s in range(B):
        g = int(gof[s])
        so = int(sof[s]) * N_ENT
        k = int(ne[s])
        pg, m2 = divmod(g, 2)
        rb = (pg % 2) * 32
        if k:
            e0 = int(ecnt[g])
            ecnt[g] += k
            idx = np.arange(e0, e0 + k)
            es, et, er = src[s, :k], tgt[s, :k], rel[s, :k]
            oh[g, so + es, idx] = 1
            oh[g, so + et, EC + idx] = 1
            oh[g, idx, 2 * EC + so + et] = 1
            relt2[pg, rb + er, m2 * EC + idx] = 1        # c=0 band
            relt2[pg, rb + 64 + er, m2 * EC + idx] = 1   # c=1 band
            np.add.at(cnt, g * P + so + et, 1.0)
        qoh[g, so + qs[s], sof[s]] = 1
        qoh[g, so + qt[s], 4 + sof[s]] = 1
    cntb = (np.asarray(inputs["msg_b2"], f).reshape(P, 1)
            * cnt[None, :]).astype(hh)

    # shared weights
    ee = np.asarray(inputs["entity_embed"], f)
    W1 = np.asarray(inputs["msg_W1"], f)
    reltab = (np.asarray(inputs["rel_embed"], f) @ W1[128:256]
              + np.asarray(inputs["msg_b1"], f))
    rt4 = np.zeros((P, 256), f)
    b2row = np.zeros((P, P), f)
    for rb in (0, 32, 64, 96):
        rt4[rb:rb + 20] = reltab
        b2row[rb] = np.asarray(inputs["msg_b2"], f)
    w2m_ = np.asarray(inputs["msg_W2"], f)
    w1u_ = np.asarray(inputs["upd_W1"], f)
    w2u_ = np.asarray(inputs["upd_W2"], f)
    cw1_ = np.asarray(inputs["cls_W1"], f)

    wf16 = np.concatenate([
        W1[0:128], W1[256:384], rt4,
        np.concatenate([w2m_[0:128], w2m_[128:256]], axis=1),
        np.concatenate(
            [w1u_[0:128, 0:128], w1u_[0:128, 128:256],
             w1u_[128:256, 0:128], w1u_[128:256, 128:256]], axis=1),
        np.concatenate([w2u_[0:128], w2u_[128:256]], axis=1),
        b2row,
    ], axis=1).astype(hh)
    wf32 = np.concatenate([
        np.asarray(inputs["upd_b1"], f).reshape(2, 128).T,
        np.asarray(inputs["upd_b2"], f).reshape(128, 1),
        np.concatenate([cw1_[0:128], cw1_[128:256]], axis=1),
        np.asarray(inputs["cls_b1"], f).reshape(128, 1),
        np.asarray(inputs["cls_W2"], f),
        np.asarray(inputs["msg_b2"], f).reshape(128, 1),
    ], axis=1).astype(f)
    shared = {
        "s0": np.tile(ee.T, (1, GRP * G)).astype(hh),
        "wf16": wf16,
        "wf32": wf32,
        "cb2": np.asarray(inputs["cls_b2"], f).reshape(20, 1).copy(),
    }

    in_maps = []
    for c in range(N_CORES):
        gsl = slice(c * G, (c + 1) * G)
        psl = slice(c * NP, (c + 1) * NP)
        m = dict(shared)
        m["oh"] = np.ascontiguousarray(
            oh[gsl].reshape(NQ, 4, P, 3 * EC).transpose(0, 2, 1, 3)
            .reshape(NQ, P, 12 * EC))
        m["relt2"] = np.ascontiguousarray(
            relt2[psl].transpose(1, 0, 2).reshape(P, NP * 256))
        m["cntb"] = np.ascontiguousarray(
            cntb[:, c * G * P:(c + 1) * G * P])
        m["qoh"] = np.ascontiguousarray(
            qoh[gsl].transpose(1, 0, 2).reshape(P, G * 8))
        in_maps.append(m)
    return in_maps, gof, sof, G


_CACHE = {}


def kernel(**inputs):
    B = np.asarray(inputs["edge_src"]).shape[0]
    in_maps, gof, sof, G = _host_prep(inputs)

    key = G
    if key not in _CACHE:
        _CACHE[key] = _build_nc(G, N_STEPS)
    nc = _CACHE[key]

    from concourse.bass_utils import run_bass_kernel_spmd
    res = run_bass_kernel_spmd(nc, in_maps, core_ids=list(range(N_CORES)))

    out = np.empty((B, N_REL), np.float32)
    allc = np.concatenate([r["out"].T for r in res.results], axis=0)
    # row index in allc: core * (G*4) + (g_local*4 + slot) = gof*4 + sof
    out[:, :] = allc[gof * 4 + sof]
    return np.ascontiguousarray(out)

